# revision 1
# baseline (speedup 1.0000x reference)
"""Cross-attention kernel for Trainium2, 8 NeuronCores SPMD.

Problem shapes (hardcoded): x [4,2048,1024], context [4,2048,1024],
mask [4,2048], HEADS=8, DIM_HEAD=64, INNER=512.

Sharding: core c handles batch b=c//2 and query-row half c%2 (1024 rows).
Each core computes all 8 heads over the full context for its rows; the
output is a disjoint [1024,1024] block -> gather is a pure concat.

Per-core dataflow (all matmul operands bf16, accumulation fp32 in PSUM):
  1. LN(x rows), LN(context) in natural layout, normalize -> bf16,
     PE-transpose 128x128 blocks -> xsT [dim,n], ctxT [dim,m] in SBUF.
  2. kT = (Wk' as lhsT).T @ ctxT   -> [inner, m]   (LN scale folded into W)
     V  = (ctxT as lhsT).T @ Wv'   -> [m, inner]   natural layout
     V_ext: per (m-chunk j, head h) slot of 65 cols = [V_h + bv | mask],
     rows scaled by mask -> masking and the softmax denominator both come
     for free out of the AV matmul.
  3. qT = (Wq' as lhsT).T @ xsT    -> [inner, n]   (q pre-scaled by d^-1/2)
  4. Attention per (head h, m-chunk j):
       simT[m128, n1024] = kT_hj.T-block @ qT_h   (PE, 2 matmuls N=512)
       pT = exp(simT)  (ACT, PSUM->SBUF bf16; no max-subtraction: logits
                        are ~N(0,1) after LN so exp cannot overflow)
       av[n128, 65] += pT-chunk.T @ V_ext_jh      (PE accumulation)
     plus null token: s0T[1,n] = k_null.T @ qT_h, e0 = exp(s0),
     av += e0-chunk.T @ [v_null | 1]  (rank-1, K=1 matmul, same PSUM group)
     Then r = 1/av[:,64] and attn_out[:, h*64:] = av[:, :64] * r.
  5. out = attn_outT @ Wo + bo -> DMA to DRAM.
"""

import numpy as np
import ml_dtypes

import concourse.bass as bass
import concourse.mybir as mybir
import concourse.tile as tile
from concourse import bacc
from concourse.masks import make_identity

F32 = mybir.dt.float32
BF16 = mybir.dt.bfloat16

P = 128
DIM = 1024
HEADS = 8
DH = 64
INNER = 512
N_CORE = 1024   # query rows per core
M = 2048        # context rows
NJ = M // P     # 16 context chunks
NQ = N_CORE // P  # 8 query chunks
KC = DIM // P   # 8 contraction chunks
EPS = 1e-6

_CACHE = {}


def build_program():
    nc = bacc.Bacc(None, target_bir_lowering=False)

    xs_d = nc.dram_tensor("xs", [N_CORE, DIM], F32, kind="ExternalInput")
    ctx_d = nc.dram_tensor("ctx", [M, DIM], F32, kind="ExternalInput")
    maskc_d = nc.dram_tensor("maskc", [P, NJ], F32, kind="ExternalInput")
    wq_d = nc.dram_tensor("wq", [DIM, INNER], BF16, kind="ExternalInput")
    wk_d = nc.dram_tensor("wk", [DIM, INNER], BF16, kind="ExternalInput")
    wv_d = nc.dram_tensor("wv", [DIM, INNER], BF16, kind="ExternalInput")
    wo_d = nc.dram_tensor("wo", [INNER, DIM], BF16, kind="ExternalInput")
    bq_d = nc.dram_tensor("bq", [1, INNER], BF16, kind="ExternalInput")
    bk_d = nc.dram_tensor("bk", [1, INNER], BF16, kind="ExternalInput")
    vb_d = nc.dram_tensor("vb", [1, INNER], BF16, kind="ExternalInput")
    bo_d = nc.dram_tensor("bo", [1, DIM], BF16, kind="ExternalInput")
    knull_d = nc.dram_tensor("knull", [P, 1], BF16, kind="ExternalInput")
    vne_d = nc.dram_tensor("vne", [1, 66], BF16, kind="ExternalInput")
    dencol_d = nc.dram_tensor("dencol", [P, NJ * HEADS], BF16, kind="ExternalInput")
    out_d = nc.dram_tensor("out", [N_CORE, DIM], F32, kind="ExternalOutput")

    with tile.TileContext(nc) as tc:
        with (
            tc.tile_pool(name="consts", bufs=1) as consts,
            tc.tile_pool(name="persist", bufs=1) as persist,
            tc.tile_pool(name="lnio", bufs=3) as lnio,
            tc.tile_pool(name="lnbf", bufs=3) as lnbf,
            tc.tile_pool(name="lntmp", bufs=4) as lntmp,
            tc.tile_pool(name="ptp", bufs=3) as ptp,
            tc.tile_pool(name="e0p", bufs=2) as e0p,
            tc.tile_pool(name="rp", bufs=2) as rp,
            tc.tile_pool(name="aop", bufs=2) as aop,
            tc.tile_pool(name="outp", bufs=2) as outp,
            tc.tile_pool(name="ps", bufs=2, space="PSUM") as psp,
            tc.tile_pool(name="av", bufs=2, space="PSUM") as avp,
        ):
            # ---- constants ----
            wq_sb = consts.tile([P, KC, INNER], BF16, tag="wq")
            nc.sync.dma_start(out=wq_sb, in_=wq_d.rearrange("(kc p) m -> p kc m", p=P))
            wk_sb = consts.tile([P, KC, INNER], BF16, tag="wk")
            nc.sync.dma_start(out=wk_sb, in_=wk_d.rearrange("(kc p) m -> p kc m", p=P))
            wv_sb = consts.tile([P, KC, INNER], BF16, tag="wv")
            nc.sync.dma_start(out=wv_sb, in_=wv_d.rearrange("(kc p) m -> p kc m", p=P))
            wo_sb = consts.tile([P, 4, DIM], BF16, tag="wo")
            nc.sync.dma_start(out=wo_sb, in_=wo_d.rearrange("(ic p) n -> p ic n", p=P))
            bqr_sb = consts.tile([1, INNER], BF16, tag="bqr")
            nc.sync.dma_start(out=bqr_sb, in_=bq_d[:])
            bkr_sb = consts.tile([1, INNER], BF16, tag="bkr")
            nc.sync.dma_start(out=bkr_sb, in_=bk_d[:])
            bvr_sb = consts.tile([1, INNER], BF16, tag="bvr")
            nc.sync.dma_start(out=bvr_sb, in_=vb_d[:])
            bor_sb = consts.tile([1, DIM], BF16, tag="bor")
            nc.sync.dma_start(out=bor_sb, in_=bo_d[:])
            ones_row = consts.tile([1, 512], BF16, tag="ones_row")
            nc.vector.memset(ones_row, 1.0)
            maskc_sb = consts.tile([P, NJ], F32, tag="maskc")
            nc.sync.dma_start(out=maskc_sb, in_=maskc_d[:])
            maskv_sb = consts.tile([P, NJ], F32, tag="maskv")
            nc.vector.tensor_copy(out=maskv_sb, in_=maskc_sb)
            knull_sb = consts.tile([P, 1], BF16, tag="knull")
            nc.sync.dma_start(out=knull_sb, in_=knull_d[:])
            vne_sb = consts.tile([1, 66], BF16, tag="vne")
            nc.sync.dma_start(out=vne_sb, in_=vne_d[:])
            ident = consts.tile([P, P], BF16, tag="ident")
            make_identity(nc, ident)
            eps_sb = consts.tile([P, 1], F32, tag="eps")
            nc.vector.memset(eps_sb, EPS)

            # ---- persistent activations ----
            ctxT = persist.tile([P, KC, M], BF16, tag="ctxT")
            xsT = persist.tile([P, KC, N_CORE], BF16, tag="xsT")
            kT = persist.tile([P, 4, M], BF16, tag="kT")
            vext = persist.tile([P, NJ, HEADS, 66], BF16, tag="vext")
            nc.sync.dma_start(out=vext[:, :, :, 64:65],
                              in_=dencol_d.rearrange("p (j h) -> p j h", j=NJ))
            qT = persist.tile([P, 4, N_CORE], BF16, tag="qT")
            attn_out = persist.tile([P, NQ, INNER], BF16, tag="attn_out")

            def ln_transpose(src_d, n_rows, dstT):
                for j in range(n_rows // P):
                    xt = lnio.tile([P, DIM], F32, tag="xt")
                    nc.sync.dma_start(out=xt, in_=src_d[j * P:(j + 1) * P, :])
                    stats = lntmp.tile([P, 2, 6], F32, tag="stats")
                    nc.vector.bn_stats(out=stats[:, 0, :], in_=xt[:, 0:512])
                    nc.vector.bn_stats(out=stats[:, 1, :], in_=xt[:, 512:1024])
                    mv = lntmp.tile([P, 2], F32, tag="mv")
                    nc.vector.bn_aggr(out=mv, in_=stats)
                    rstd = lntmp.tile([P, 1], F32, tag="rstd")
                    nc.scalar.activation(out=rstd, in_=mv[:, 1:2],
                                         func=mybir.ActivationFunctionType.Sqrt,
                                         bias=eps_sb)
                    nc.vector.reciprocal(out=rstd, in_=rstd)
                    xn = lnbf.tile([P, DIM], BF16, tag="xn")
                    nc.vector.tensor_scalar(
                        out=xn, in0=xt, scalar1=mv[:, 0:1], scalar2=rstd,
                        op0=mybir.AluOpType.subtract, op1=mybir.AluOpType.mult)
                    tp = psp.tile([P, KC * P], BF16, tag="ps")
                    for i in range(KC):
                        nc.tensor.transpose(out=tp[:, i * P:(i + 1) * P],
                                            in_=xn[:, i * P:(i + 1) * P],
                                            identity=ident)
                    for i in range(KC):
                        nc.scalar.copy(out=dstT[:, i, j * P:(j + 1) * P],
                                       in_=tp[:, i * P:(i + 1) * P])

            ln_transpose(ctx_d, M, ctxT)
            ln_transpose(xs_d, N_CORE, xsT)

            # ---- kT projection: [inner, m] ----
            for ic in range(4):
                for mh in range(4):
                    ps = psp.tile([P, 512], F32, tag="ps")
                    for kc in range(KC):
                        nc.tensor.matmul(
                            out=ps,
                            lhsT=wk_sb[:, kc, ic * P:(ic + 1) * P],
                            rhs=ctxT[:, kc, mh * 512:(mh + 1) * 512],
                            start=(kc == 0), stop=False)
                    nc.tensor.matmul(
                        out=ps, lhsT=bkr_sb[:, ic * P:(ic + 1) * P],
                        rhs=ones_row, start=False, stop=True)
                    nc.vector.tensor_copy(
                        out=kT[:, ic, mh * 512:(mh + 1) * 512], in_=ps)

            # ---- V projection (natural layout) + mask/bias -> V_ext ----
            for j in range(NJ):
                ps = avp.tile([P, 512], F32, tag="av")
                for kc in range(KC):
                    nc.tensor.matmul(
                        out=ps,
                        lhsT=ctxT[:, kc, j * P:(j + 1) * P],
                        rhs=wv_sb[:, kc, :],
                        start=(kc == 0), stop=False)
                nc.tensor.matmul(
                    out=ps, lhsT=ones_row[:, 0:P], rhs=bvr_sb,
                    start=False, stop=True)
                for h in range(HEADS):
                    nc.vector.tensor_scalar_mul(
                        out=vext[:, j, h, 0:64],
                        in0=ps[:, h * 64:(h + 1) * 64],
                        scalar1=maskv_sb[:, j:j + 1])

            # ---- q projection: [inner, n] ----
            for ic in range(4):
                for nh in range(2):
                    ps = psp.tile([P, 512], F32, tag="ps")
                    for kc in range(KC):
                        nc.tensor.matmul(
                            out=ps,
                            lhsT=wq_sb[:, kc, ic * P:(ic + 1) * P],
                            rhs=xsT[:, kc, nh * 512:(nh + 1) * 512],
                            start=(kc == 0), stop=False)
                    nc.tensor.matmul(
                        out=ps, lhsT=bqr_sb[:, ic * P:(ic + 1) * P],
                        rhs=ones_row, start=False, stop=True)
                    nc.vector.tensor_copy(
                        out=qT[:, ic, nh * 512:(nh + 1) * 512], in_=ps)

            # ---- attention ----
            for h in range(HEADS):
                hp = (h % 2) * DH
                ic = h // 2
                qh = qT[hp:hp + DH, ic, :]
                # null-token logits s0T[1, n] and e0 = exp(s0)
                s0 = psp.tile([1, N_CORE], F32, tag="ps")
                nc.tensor.matmul(out=s0[:, 0:512], lhsT=knull_sb[hp:hp + DH, :],
                                 rhs=qh[:, 0:512], start=True, stop=True)
                nc.tensor.matmul(out=s0[:, 512:1024], lhsT=knull_sb[hp:hp + DH, :],
                                 rhs=qh[:, 512:1024], start=True, stop=True)
                e0 = e0p.tile([1, N_CORE], BF16, tag="e0")
                nc.scalar.activation(out=e0, in_=s0,
                                     func=mybir.ActivationFunctionType.Exp)
                av = avp.tile([P, NQ, P], F32, tag="av")
                # PSUM start_tensor_calc zeroes a whole 2KB bank (4 of our
                # 128-f32 slots), so only the first matmul touching each bank
                # carries start=True; every slot's first write then lands on
                # still-pending-zero bytes and overwrites, later ones
                # accumulate. Group bookkeeping is bank-granular, hence
                # skip_group_check. The null-token rank-1 matmul opens each
                # slot (e0 is ready before the j loop).
                for q4 in range(NQ):
                    nc.tensor.matmul(
                        out=av[:, q4, 0:65],
                        lhsT=e0[:, q4 * P:(q4 + 1) * P],
                        rhs=vne_sb[:, 0:65],
                        start=(q4 % 4 == 0), stop=False,
                        skip_group_check=True)
                for j in range(NJ):
                    sm = psp.tile([P, N_CORE], F32, tag="ps")
                    kh = kT[hp:hp + DH, ic, j * P:(j + 1) * P]
                    nc.tensor.matmul(out=sm[:, 0:512], lhsT=kh, rhs=qh[:, 0:512],
                                     start=True, stop=True)
                    nc.tensor.matmul(out=sm[:, 512:1024], lhsT=kh,
                                     rhs=qh[:, 512:1024], start=True, stop=True)
                    pt = ptp.tile([P, N_CORE], BF16, tag="pt")
                    nc.scalar.activation(out=pt, in_=sm,
                                         func=mybir.ActivationFunctionType.Exp)
                    for q4 in range(NQ):
                        nc.tensor.matmul(
                            out=av[:, q4, 0:65],
                            lhsT=pt[:, q4 * P:(q4 + 1) * P],
                            rhs=vext[:, j, h, 0:65],
                            start=False, stop=(j == NJ - 1 and q4 % 4 == 3),
                            skip_group_check=True)
                r = rp.tile([P, NQ], F32, tag="r")
                for q4 in range(NQ):
                    nc.vector.reciprocal(out=r[:, q4:q4 + 1],
                                         in_=av[:, q4, 64:65])
                for q4 in range(NQ):
                    nc.vector.tensor_scalar_mul(
                        out=attn_out[:, q4, h * DH:(h + 1) * DH],
                        in0=av[:, q4, 0:64], scalar1=r[:, q4:q4 + 1])

            # ---- output projection ----
            for q4 in range(NQ):
                tp = psp.tile([P, 4 * P], BF16, tag="ps")
                for i in range(4):
                    nc.tensor.transpose(out=tp[:, i * P:(i + 1) * P],
                                        in_=attn_out[:, q4, i * P:(i + 1) * P],
                                        identity=ident)
                aoT = aop.tile([P, 4 * P], BF16, tag="aoT")
                nc.vector.tensor_copy(out=aoT, in_=tp)
                ot = outp.tile([P, DIM], F32, tag="ot")
                for oh in range(2):
                    ps = avp.tile([P, 512], F32, tag="av")
                    for ic in range(4):
                        nc.tensor.matmul(
                            out=ps, lhsT=aoT[:, ic * P:(ic + 1) * P],
                            rhs=wo_sb[:, ic, oh * 512:(oh + 1) * 512],
                            start=(ic == 0), stop=False)
                    nc.tensor.matmul(
                        out=ps, lhsT=ones_row[:, 0:P],
                        rhs=bor_sb[:, oh * 512:(oh + 1) * 512],
                        start=False, stop=True)
                    nc.vector.tensor_copy(
                        out=ot[:, oh * 512:(oh + 1) * 512], in_=ps)
                nc.sync.dma_start(out=out_d[q4 * P:(q4 + 1) * P, :], in_=ot)

    nc.compile()
    return nc


def prep_inputs(x, context, mask, ln_x_scale, ln_x_bias, ln_c_scale, ln_c_bias,
                Wq, bq, Wkv, bkv, Wo, bo, null_kv):
    """Host-side weight folding + per-core input maps."""
    f32 = np.float32
    bf16 = ml_dtypes.bfloat16
    scale = np.float32(DH ** (-0.5))
    x = np.asarray(x, f32)
    context = np.asarray(context, f32)
    mask = np.asarray(mask)
    Wq = np.asarray(Wq, f32)
    Wkv = np.asarray(Wkv, f32)
    Wo = np.asarray(Wo, f32)
    ln_x_scale = np.asarray(ln_x_scale, f32)
    ln_x_bias = np.asarray(ln_x_bias, f32)
    ln_c_scale = np.asarray(ln_c_scale, f32)
    ln_c_bias = np.asarray(ln_c_bias, f32)
    bq = np.asarray(bq, f32)
    bkv = np.asarray(bkv, f32)
    bo = np.asarray(bo, f32)
    null_kv = np.asarray(null_kv, f32)

    wq_f = (ln_x_scale[:, None] * Wq) * scale
    bq_f = (ln_x_bias @ Wq + bq) * scale
    wkv_f = ln_c_scale[:, None] * Wkv
    bkv_f = ln_c_bias @ Wkv + bkv
    wk_f, wv_f = wkv_f[:, :INNER], wkv_f[:, INNER:]
    bk_f, bv_f = bkv_f[:INNER], bkv_f[INNER:]

    shared = {
        "wq": np.ascontiguousarray(wq_f.astype(bf16)),
        "wk": np.ascontiguousarray(wk_f.astype(bf16)),
        "wv": np.ascontiguousarray(wv_f.astype(bf16)),
        "wo": np.ascontiguousarray(Wo.astype(bf16)),
        "bq": np.ascontiguousarray(bq_f.reshape(1, INNER).astype(bf16)),
        "bk": np.ascontiguousarray(bk_f.reshape(1, INNER).astype(bf16)),
        "vb": np.ascontiguousarray(bv_f.reshape(1, INNER).astype(bf16)),
        "bo": np.ascontiguousarray(bo.reshape(1, DIM).astype(bf16)),
        "knull": np.ascontiguousarray(
            np.tile(null_kv[0], 2).reshape(P, 1).astype(bf16)),
        "vne": np.ascontiguousarray(
            np.concatenate([null_kv[1], [1.0, 0.0]]).reshape(1, 66).astype(bf16)),
    }
    in_maps = []
    for c in range(8):
        b, half = c // 2, c % 2
        maskc = mask[b].astype(f32).reshape(NJ, P).T
        in_maps.append({
            "xs": np.ascontiguousarray(x[b, half * N_CORE:(half + 1) * N_CORE]),
            "ctx": np.ascontiguousarray(context[b]),
            "maskc": np.ascontiguousarray(maskc),
            "dencol": np.ascontiguousarray(np.repeat(maskc, HEADS, axis=1).astype(bf16)),
            **shared,
        })
    return in_maps


def kernel(**inputs):
    from concourse.bass_utils import run_bass_kernel_spmd

    if "nc" not in _CACHE:
        _CACHE["nc"] = build_program()
    nc = _CACHE["nc"]
    in_maps = prep_inputs(**inputs)
    res = run_bass_kernel_spmd(nc, in_maps, list(range(8)))
    out = np.empty((4, 2048, DIM), np.float32)
    for c in range(8):
        b, half = c // 2, c % 2
        out[b, half * N_CORE:(half + 1) * N_CORE] = res.results[c]["out"]
    return out



# revision 6
# speedup vs baseline: 4.4838x; 4.4838x over previous
"""Cross-attention kernel for Trainium2, 8 NeuronCores SPMD.

Problem shapes (hardcoded): x [4,2048,1024], context [4,2048,1024],
mask [4,2048], HEADS=8, DIM_HEAD=64, INNER=512.

Host<->device traffic is the bottleneck (axon-tunneled devices, ~55 MB/s,
full duplex), so the runtime is organized around the wire:
  - x and context cross the wire once each, as fp16 (16.8 MB apiece).
  - The output returns as fp16 (16.8 MB), upcast to f32 on host.
  - Weights are folded (LN scales into W, q pre-scaled) and cached on
    device across calls; donation zero-buffers are created on device.
  - Work is pipelined in 4 stages (one per batch): while stage b executes,
    stage b+1's inputs upload, and finished outputs stream back on reader
    threads (d2h overlaps h2d).

Per stage, all 8 cores cooperate on one batch: core c owns query rows
[c*256,(c+1)*256) and contributes ctx rows [c*256,(c+1)*256). An in-kernel
AllGather (DRAM bounce buffers, gpsimd) replicates the raw fp16 ctx shard
to all cores, so each context byte crosses the tunnel exactly once; every
core then LNs/projects the full 2048-row context locally (compute is far
from the roofline, redundancy is free) and runs attention for its 256
queries over all 8 heads.

Per-core dataflow (matmul operands bf16, accumulation fp32 in PSUM):
  1. LN(xs rows) -> bf16, PE-transpose -> xsT [dim, 256]; qT = Wq'.T @ xsT
     (LN scale folded into W, q pre-scaled by d^-1/2).
  2. AllGather ctx -> ctx_g [2048, dim] fp16; LN -> ctxT [dim, 2048].
  3. kT = Wk'.T @ ctxT -> [inner, 2048]; V = ctxT.T @ Wv' -> [2048, inner];
     V_ext per (j, h): 66-col slot [V_h*mask | mask | pad] -> masking and
     the softmax denominator both come free out of the AV matmul.
  4. Attention per (head h, m-chunk j):
       simT[m128, 256] = kT_hj.T @ qT_h   (PE)
       pT = exp(simT)  (ACT, PSUM->SBUF bf16; logits ~N(0,1) after LN so
                        exp cannot overflow without max-subtraction)
       av[q128, 65] += pT.T @ V_ext_jh    (PE accumulation)
     plus null token: s0 = k_null.T @ qT_h, e0 = exp(s0),
     av += e0.T @ [v_null | 1] (rank-1 matmul opens the PSUM bank).
     r = 1/av[:,64]; attn_out[:, h*64:] = av[:, :64] * r.
  5. out = attn_outT @ Wo + bo -> fp16 -> DMA to DRAM.
"""

import threading
import zlib

import numpy as np
import ml_dtypes

import jax
import jax.numpy as jnp
from jax.sharding import Mesh, PartitionSpec, NamedSharding
from jax.experimental.shard_map import shard_map

import concourse.bass as bass
import concourse.mybir as mybir
import concourse.tile as tile
from concourse import bacc
from concourse.masks import make_identity

F32 = mybir.dt.float32
F16 = mybir.dt.float16
BF16 = mybir.dt.bfloat16

P = 128
DIM = 1024
HEADS = 8
DH = 64
INNER = 512
B = 4
N = 2048          # query rows per batch
M = 2048          # context rows per batch
NQR = 256         # query rows per core per stage
NCR = 256         # ctx rows contributed per core per stage
NQ = NQR // P     # 2 query chunks
NJ = M // P       # 16 context chunks
KC = DIM // P     # 8 contraction chunks
EPS = 1e-6
NCORES = 8

_RT = {}


def build_program():
    nc = bacc.Bacc(None, target_bir_lowering=False, num_devices=NCORES)

    xs_d = nc.dram_tensor("xs", [NQR, DIM], F16, kind="ExternalInput")
    ctxs_d = nc.dram_tensor("ctxs", [NCR, DIM], F16, kind="ExternalInput")
    maskm_d = nc.dram_tensor("maskm", [P, NJ], F32, kind="ExternalInput")
    wq_d = nc.dram_tensor("wq", [DIM, INNER], BF16, kind="ExternalInput")
    wk_d = nc.dram_tensor("wk", [DIM, INNER], BF16, kind="ExternalInput")
    wv_d = nc.dram_tensor("wv", [DIM, INNER], BF16, kind="ExternalInput")
    wo_d = nc.dram_tensor("wo", [INNER, DIM], BF16, kind="ExternalInput")
    bq_d = nc.dram_tensor("bq", [1, INNER], BF16, kind="ExternalInput")
    bk_d = nc.dram_tensor("bk", [1, INNER], BF16, kind="ExternalInput")
    vb_d = nc.dram_tensor("vb", [1, INNER], BF16, kind="ExternalInput")
    bo_d = nc.dram_tensor("bo", [1, DIM], BF16, kind="ExternalInput")
    knull_d = nc.dram_tensor("knull", [P, 1], BF16, kind="ExternalInput")
    vne_d = nc.dram_tensor("vne", [1, 66], BF16, kind="ExternalInput")
    out_d = nc.dram_tensor("out", [NQR, DIM], F16, kind="ExternalOutput")

    with tile.TileContext(nc) as tc:
        with (
            tc.tile_pool(name="dram", bufs=1, space="DRAM") as dram,
            tc.tile_pool(name="consts", bufs=1) as consts,
            tc.tile_pool(name="persist", bufs=1) as persist,
            tc.tile_pool(name="lnio", bufs=3) as lnio,
            tc.tile_pool(name="lnbf", bufs=3) as lnbf,
            tc.tile_pool(name="lntmp", bufs=4) as lntmp,
            tc.tile_pool(name="ptp", bufs=3) as ptp,
            tc.tile_pool(name="e0p", bufs=2) as e0p,
            tc.tile_pool(name="rp", bufs=2) as rp,
            tc.tile_pool(name="aop", bufs=2) as aop,
            tc.tile_pool(name="outp", bufs=2) as outp,
            tc.tile_pool(name="ps", bufs=2, space="PSUM") as psp,
            tc.tile_pool(name="sim", bufs=2, space="PSUM") as simp,
            tc.tile_pool(name="av", bufs=2, space="PSUM") as avp,
        ):
            # ---- ctx AllGather (gpsimd, DRAM bounce) ----
            ctx_bounce = dram.tile([NCR, DIM], F16, tag="ctxb")
            ctx_g = dram.tile([M, DIM], F16, tag="ctxg")
            nc.gpsimd.dma_start(ctx_bounce[:], ctxs_d[:])
            nc.gpsimd.collective_compute(
                "AllGather",
                mybir.AluOpType.bypass,
                replica_groups=[list(range(NCORES))],
                ins=[ctx_bounce.opt()],
                outs=[ctx_g.opt()],
            )

            # ---- constants ----
            wq_sb = consts.tile([P, KC, INNER], BF16, tag="wq")
            nc.sync.dma_start(out=wq_sb, in_=wq_d.rearrange("(kc p) m -> p kc m", p=P))
            wk_sb = consts.tile([P, KC, INNER], BF16, tag="wk")
            nc.sync.dma_start(out=wk_sb, in_=wk_d.rearrange("(kc p) m -> p kc m", p=P))
            wv_sb = consts.tile([P, KC, INNER], BF16, tag="wv")
            nc.sync.dma_start(out=wv_sb, in_=wv_d.rearrange("(kc p) m -> p kc m", p=P))
            wo_sb = consts.tile([P, 4, DIM], BF16, tag="wo")
            nc.sync.dma_start(out=wo_sb, in_=wo_d.rearrange("(ic p) n -> p ic n", p=P))
            bqr_sb = consts.tile([1, INNER], BF16, tag="bqr")
            nc.sync.dma_start(out=bqr_sb, in_=bq_d[:])
            bkr_sb = consts.tile([1, INNER], BF16, tag="bkr")
            nc.sync.dma_start(out=bkr_sb, in_=bk_d[:])
            bvr_sb = consts.tile([1, INNER], BF16, tag="bvr")
            nc.sync.dma_start(out=bvr_sb, in_=vb_d[:])
            bor_sb = consts.tile([1, DIM], BF16, tag="bor")
            nc.sync.dma_start(out=bor_sb, in_=bo_d[:])
            ones_row = consts.tile([1, 512], BF16, tag="ones_row")
            nc.vector.memset(ones_row, 1.0)
            maskm_sb = consts.tile([P, NJ], F32, tag="maskm")
            nc.sync.dma_start(out=maskm_sb, in_=maskm_d[:])
            knull_sb = consts.tile([P, 1], BF16, tag="knull")
            nc.sync.dma_start(out=knull_sb, in_=knull_d[:])
            vne_sb = consts.tile([1, 66], BF16, tag="vne")
            nc.sync.dma_start(out=vne_sb, in_=vne_d[:])
            ident = consts.tile([P, P], BF16, tag="ident")
            make_identity(nc, ident)
            eps_sb = consts.tile([P, 1], F32, tag="eps")
            nc.vector.memset(eps_sb, EPS)

            # ---- persistent activations ----
            ctxT = persist.tile([P, KC, M], BF16, tag="ctxT")
            xsT = persist.tile([P, KC, NQR], BF16, tag="xsT")
            kT = persist.tile([P, 4, M], BF16, tag="kT")
            vext = persist.tile([P, NJ, HEADS, 66], BF16, tag="vext")
            qT = persist.tile([P, 4, NQR], BF16, tag="qT")
            attn_out = persist.tile([P, NQ, INNER], BF16, tag="attn_out")

            def ln_transpose(src, n_rows, dstT):
                for j in range(n_rows // P):
                    xt = lnio.tile([P, DIM], F16, tag="xt")
                    nc.sync.dma_start(out=xt, in_=src[j * P:(j + 1) * P, :])
                    stats = lntmp.tile([P, 2, 6], F32, tag="stats")
                    nc.vector.bn_stats(out=stats[:, 0, :], in_=xt[:, 0:512])
                    nc.vector.bn_stats(out=stats[:, 1, :], in_=xt[:, 512:1024])
                    mv = lntmp.tile([P, 2], F32, tag="mv")
                    nc.vector.bn_aggr(out=mv, in_=stats)
                    rstd = lntmp.tile([P, 1], F32, tag="rstd")
                    nc.scalar.activation(out=rstd, in_=mv[:, 1:2],
                                         func=mybir.ActivationFunctionType.Sqrt,
                                         bias=eps_sb)
                    nc.vector.reciprocal(out=rstd, in_=rstd)
                    xn = lnbf.tile([P, DIM], BF16, tag="xn")
                    nc.vector.tensor_scalar(
                        out=xn, in0=xt, scalar1=mv[:, 0:1], scalar2=rstd,
                        op0=mybir.AluOpType.subtract, op1=mybir.AluOpType.mult)
                    tp = psp.tile([P, KC * P], BF16, tag="ps")
                    for i in range(KC):
                        nc.tensor.transpose(out=tp[:, i * P:(i + 1) * P],
                                            in_=xn[:, i * P:(i + 1) * P],
                                            identity=ident)
                    for i in range(KC):
                        nc.scalar.copy(out=dstT[:, i, j * P:(j + 1) * P],
                                       in_=tp[:, i * P:(i + 1) * P])

            # queries first: independent of the collective
            ln_transpose(xs_d, NQR, xsT)

            # ---- q projection: [inner, 256] ----
            for ic in range(4):
                ps = psp.tile([P, 512], F32, tag="ps")
                for kc in range(KC):
                    nc.tensor.matmul(
                        out=ps[:, 0:NQR],
                        lhsT=wq_sb[:, kc, ic * P:(ic + 1) * P],
                        rhs=xsT[:, kc, :],
                        start=(kc == 0), stop=False)
                nc.tensor.matmul(
                    out=ps[:, 0:NQR], lhsT=bqr_sb[:, ic * P:(ic + 1) * P],
                    rhs=ones_row[:, 0:NQR], start=False, stop=True)
                nc.vector.tensor_copy(out=qT[:, ic, :], in_=ps[:, 0:NQR])

            # ---- gathered context: LN + transpose ----
            ln_transpose(ctx_g, M, ctxT)

            # ---- kT projection: [inner, m] ----
            for ic in range(4):
                for mh in range(4):
                    ps = psp.tile([P, 512], F32, tag="ps")
                    for kc in range(KC):
                        nc.tensor.matmul(
                            out=ps,
                            lhsT=wk_sb[:, kc, ic * P:(ic + 1) * P],
                            rhs=ctxT[:, kc, mh * 512:(mh + 1) * 512],
                            start=(kc == 0), stop=False)
                    nc.tensor.matmul(
                        out=ps, lhsT=bkr_sb[:, ic * P:(ic + 1) * P],
                        rhs=ones_row, start=False, stop=True)
                    nc.vector.tensor_copy(
                        out=kT[:, ic, mh * 512:(mh + 1) * 512], in_=ps)

            # ---- V projection (natural layout) + mask/bias -> V_ext ----
            for j in range(NJ):
                ps = psp.tile([P, 512], F32, tag="ps")
                for kc in range(KC):
                    nc.tensor.matmul(
                        out=ps,
                        lhsT=ctxT[:, kc, j * P:(j + 1) * P],
                        rhs=wv_sb[:, kc, :],
                        start=(kc == 0), stop=False)
                nc.tensor.matmul(
                    out=ps, lhsT=ones_row[:, 0:P], rhs=bvr_sb,
                    start=False, stop=True)
                for h in range(HEADS):
                    nc.vector.tensor_scalar_mul(
                        out=vext[:, j, h, 0:64],
                        in0=ps[:, h * 64:(h + 1) * 64],
                        scalar1=maskm_sb[:, j:j + 1])
                # denominator column: mask value (0/1) per row
                for h in range(HEADS):
                    nc.scalar.copy(out=vext[:, j, h, 64:65],
                                   in_=maskm_sb[:, j:j + 1])

            # ---- attention ----
            for h in range(HEADS):
                hp = (h % 2) * DH
                ic = h // 2
                qh = qT[hp:hp + DH, ic, :]
                # null-token logits s0[1, 256] and e0 = exp(s0)
                s0 = psp.tile([1, 512], F32, tag="s0")
                nc.tensor.matmul(out=s0[:, 0:NQR], lhsT=knull_sb[hp:hp + DH, :],
                                 rhs=qh, start=True, stop=True)
                e0 = e0p.tile([1, NQR], BF16, tag="e0")
                nc.scalar.activation(out=e0, in_=s0[:, 0:NQR],
                                     func=mybir.ActivationFunctionType.Exp)
                # av [P, 4, P] f32 = exactly one 2KB PSUM bank; slots 0..1
                # used. start=True on the first (null) matmul zeroes the
                # bank; all later matmuls accumulate (bank-granular
                # bookkeeping, hence skip_group_check).
                av = avp.tile([P, 4, P], F32, tag="av")
                for q2 in range(NQ):
                    nc.tensor.matmul(
                        out=av[:, q2, 0:65],
                        lhsT=e0[:, q2 * P:(q2 + 1) * P],
                        rhs=vne_sb[:, 0:65],
                        start=(q2 == 0), stop=False,
                        skip_group_check=True)
                for j in range(NJ):
                    sm = simp.tile([P, 512], F32, tag="sim")
                    kh = kT[hp:hp + DH, ic, j * P:(j + 1) * P]
                    nc.tensor.matmul(out=sm[:, 0:NQR], lhsT=kh, rhs=qh,
                                     start=True, stop=True)
                    pt = ptp.tile([P, NQR], BF16, tag="pt")
                    nc.scalar.activation(out=pt, in_=sm[:, 0:NQR],
                                         func=mybir.ActivationFunctionType.Exp)
                    for q2 in range(NQ):
                        nc.tensor.matmul(
                            out=av[:, q2, 0:65],
                            lhsT=pt[:, q2 * P:(q2 + 1) * P],
                            rhs=vext[:, j, h, 0:65],
                            start=False, stop=(j == NJ - 1 and q2 == NQ - 1),
                            skip_group_check=True)
                r = rp.tile([P, NQ], F32, tag="r")
                for q2 in range(NQ):
                    nc.vector.reciprocal(out=r[:, q2:q2 + 1],
                                         in_=av[:, q2, 64:65])
                for q2 in range(NQ):
                    nc.vector.tensor_scalar_mul(
                        out=attn_out[:, q2, h * DH:(h + 1) * DH],
                        in0=av[:, q2, 0:64], scalar1=r[:, q2:q2 + 1])

            # ---- output projection ----
            for q2 in range(NQ):
                tp = psp.tile([P, KC * P], BF16, tag="ps")
                for i in range(4):
                    nc.tensor.transpose(out=tp[:, i * P:(i + 1) * P],
                                        in_=attn_out[:, q2, i * P:(i + 1) * P],
                                        identity=ident)
                aoT = aop.tile([P, 4 * P], BF16, tag="aoT")
                nc.vector.tensor_copy(out=aoT, in_=tp[:, 0:4 * P])
                ot = outp.tile([P, DIM], F16, tag="ot")
                for oh in range(2):
                    ps = psp.tile([P, 512], F32, tag="ps")
                    for ic in range(4):
                        nc.tensor.matmul(
                            out=ps, lhsT=aoT[:, ic * P:(ic + 1) * P],
                            rhs=wo_sb[:, ic, oh * 512:(oh + 1) * 512],
                            start=(ic == 0), stop=False)
                    nc.tensor.matmul(
                        out=ps, lhsT=ones_row[:, 0:P],
                        rhs=bor_sb[:, oh * 512:(oh + 1) * 512],
                        start=False, stop=True)
                    nc.vector.tensor_copy(
                        out=ot[:, oh * 512:(oh + 1) * 512], in_=ps)
                nc.sync.dma_start(out=out_d[q2 * P:(q2 + 1) * P, :], in_=ot)

    nc.compile()
    return nc


def _build_runtime():
    from concourse.bass2jax import (
        _bass_exec_p, install_neuronx_cc_hook, partition_id_tensor)

    nc = build_program()
    install_neuronx_cc_hook()
    partition_name = nc.partition_id_tensor.name if nc.partition_id_tensor else None
    in_names, out_names, out_avals = [], [], []
    for alloc in nc.m.functions[0].allocations:
        if not isinstance(alloc, mybir.MemoryLocationSet):
            continue
        name = alloc.memorylocations[0].name
        if alloc.kind == "ExternalInput":
            if name != partition_name:
                in_names.append(name)
        elif alloc.kind == "ExternalOutput":
            out_names.append(name)
            out_avals.append(jax.core.ShapedArray(
                tuple(alloc.tensor_shape), mybir.dt.np(alloc.dtype)))
    n_params = len(in_names)
    n_outs = len(out_avals)
    in_names_all = in_names + out_names + (
        [partition_name] if partition_name else [])
    donate = tuple(range(n_params, n_params + n_outs))

    def _body(*args):
        operands = list(args)
        if partition_name is not None:
            operands.append(partition_id_tensor())
        return tuple(_bass_exec_p.bind(
            *operands, out_avals=tuple(out_avals), in_names=tuple(in_names_all),
            out_names=tuple(out_names), lowering_input_output_aliases=(),
            sim_require_finite=True, sim_require_nnan=True, nc=nc))

    devices = jax.devices()[:NCORES]
    mesh = Mesh(np.asarray(devices), ("core",))
    S = NamedSharding(mesh, PartitionSpec("core"))
    sharded = jax.jit(
        shard_map(_body, mesh=mesh,
                  in_specs=(PartitionSpec("core"),) * (n_params + n_outs),
                  out_specs=(PartitionSpec("core"),) * n_outs,
                  check_rep=False),
        donate_argnums=donate, keep_unused=True)

    zeros_jit = jax.jit(
        lambda: tuple(
            jnp.zeros((NCORES * a.shape[0], *a.shape[1:]), a.dtype)
            for a in out_avals),
        out_shardings=tuple([S] * n_outs))

    return dict(nc=nc, sharded=sharded, zeros_jit=zeros_jit,
                in_names=in_names, sharding=S)


def _fold_weights(ln_x_scale, ln_x_bias, ln_c_scale, ln_c_bias,
                  Wq, bq, Wkv, bkv, Wo, bo, null_kv):
    f32 = np.float32
    bf16 = ml_dtypes.bfloat16
    scale = np.float32(DH ** (-0.5))
    Wq = np.asarray(Wq, f32)
    Wkv = np.asarray(Wkv, f32)
    Wo = np.asarray(Wo, f32)
    ln_x_scale = np.asarray(ln_x_scale, f32)
    ln_x_bias = np.asarray(ln_x_bias, f32)
    ln_c_scale = np.asarray(ln_c_scale, f32)
    ln_c_bias = np.asarray(ln_c_bias, f32)
    bq = np.asarray(bq, f32)
    bkv = np.asarray(bkv, f32)
    bo = np.asarray(bo, f32)
    null_kv = np.asarray(null_kv, f32)

    wq_f = (ln_x_scale[:, None] * Wq) * scale
    bq_f = (ln_x_bias @ Wq + bq) * scale
    wkv_f = ln_c_scale[:, None] * Wkv
    bkv_f = ln_c_bias @ Wkv + bkv
    wk_f, wv_f = wkv_f[:, :INNER], wkv_f[:, INNER:]
    bk_f, bv_f = bkv_f[:INNER], bkv_f[INNER:]

    return {
        "wq": np.ascontiguousarray(wq_f.astype(bf16)),
        "wk": np.ascontiguousarray(wk_f.astype(bf16)),
        "wv": np.ascontiguousarray(wv_f.astype(bf16)),
        "wo": np.ascontiguousarray(Wo.astype(bf16)),
        "bq": np.ascontiguousarray(bq_f.reshape(1, INNER).astype(bf16)),
        "bk": np.ascontiguousarray(bk_f.reshape(1, INNER).astype(bf16)),
        "vb": np.ascontiguousarray(bv_f.reshape(1, INNER).astype(bf16)),
        "bo": np.ascontiguousarray(bo.reshape(1, DIM).astype(bf16)),
        "knull": np.ascontiguousarray(
            np.tile(null_kv[0], 2).reshape(P, 1).astype(bf16)),
        "vne": np.ascontiguousarray(
            np.concatenate([null_kv[1], [1.0, 0.0]]).reshape(1, 66).astype(bf16)),
    }


def _weights_key(*arrs):
    h = 0
    for a in arrs:
        a = np.ascontiguousarray(a)
        h = zlib.adler32(a.view(np.uint8).reshape(-1), h)
    return h


def kernel(**inputs):
    if "rt" not in _RT:
        _RT["rt"] = _build_runtime()
    rt = _RT["rt"]
    S = rt["sharding"]

    x = np.asarray(inputs["x"], np.float32)
    context = np.asarray(inputs["context"], np.float32)
    mask = np.asarray(inputs["mask"])

    wkey = _weights_key(
        np.asarray(inputs["Wq"], np.float32), np.asarray(inputs["Wkv"], np.float32),
        np.asarray(inputs["Wo"], np.float32), np.asarray(inputs["bq"], np.float32),
        np.asarray(inputs["bkv"], np.float32), np.asarray(inputs["bo"], np.float32),
        np.asarray(inputs["ln_x_scale"], np.float32), np.asarray(inputs["ln_x_bias"], np.float32),
        np.asarray(inputs["ln_c_scale"], np.float32), np.asarray(inputs["ln_c_bias"], np.float32),
        np.asarray(inputs["null_kv"], np.float32))
    if _RT.get("wkey") != wkey:
        shared = _fold_weights(
            inputs["ln_x_scale"], inputs["ln_x_bias"], inputs["ln_c_scale"],
            inputs["ln_c_bias"], inputs["Wq"], inputs["bq"], inputs["Wkv"],
            inputs["bkv"], inputs["Wo"], inputs["bo"], inputs["null_kv"])
        dev_w = {}
        for name, arr in shared.items():
            cat = np.ascontiguousarray(
                np.tile(arr, (NCORES,) + (1,) * (arr.ndim - 1)))
            dev_w[name] = jax.device_put(cat, S)
        jax.block_until_ready(list(dev_w.values()))
        _RT["dev_w"] = dev_w
        _RT["wkey"] = wkey
    dev_w = _RT["dev_w"]

    x16 = x.astype(np.float16)
    c16 = context.astype(np.float16)
    maskf = mask.astype(ml_dtypes.bfloat16)

    out = np.empty((B, N, DIM), np.float32)
    results = [None] * B
    threads = []

    def fetch(b, arr):
        results[b] = np.asarray(arr)

    in_names = rt["in_names"]
    for b in range(B):
        # per-core shards are contiguous row blocks: concat == the batch slab
        d_x = jax.device_put(x16[b], S)
        d_c = jax.device_put(c16[b], S)
        maskc = np.ascontiguousarray(
            np.tile(mask[b].astype(np.float32).reshape(NJ, P).T, (NCORES, 1)))
        d_m = jax.device_put(maskc, S)
        stage_in = {"xs": d_x, "ctxs": d_c, "maskm": d_m, **dev_w}
        args = [stage_in[name] for name in in_names]
        zeros = rt["zeros_jit"]()
        outs = rt["sharded"](*args, *zeros)
        th = threading.Thread(target=fetch, args=(b, outs[0]))
        th.start()
        threads.append(th)

    for th in threads:
        th.join()
    for b in range(B):
        out[b] = results[b].astype(np.float32)
    return out


# revision 7
# speedup vs baseline: 4.8740x; 1.0870x over previous
"""Cross-attention kernel for Trainium2, 8 NeuronCores SPMD.

Problem shapes (hardcoded): x [4,2048,1024], context [4,2048,1024],
mask [4,2048], HEADS=8, DIM_HEAD=64, INNER=512.

Host<->device traffic is the bottleneck (axon-tunneled devices, ~55 MB/s,
full duplex), so the runtime is organized around the wire:
  - x and context cross the wire once each, as fp16 (16.8 MB apiece).
  - The output returns as fp16 (16.8 MB), upcast to f32 on host.
  - Weights are folded (LN scales into W, q pre-scaled) and cached on
    device across calls; donation zero-buffers are created on device.
  - Work is pipelined in 4 stages (one per batch): while stage b executes,
    stage b+1's inputs upload, and finished outputs stream back on reader
    threads (d2h overlaps h2d).

Per stage, all 8 cores cooperate on one batch: core c owns query rows
[c*256,(c+1)*256) and contributes ctx rows [c*256,(c+1)*256). An in-kernel
AllGather (DRAM bounce buffers, gpsimd) replicates the raw fp16 ctx shard
to all cores, so each context byte crosses the tunnel exactly once; every
core then LNs/projects the full 2048-row context locally (compute is far
from the roofline, redundancy is free) and runs attention for its 256
queries over all 8 heads.

Per-core dataflow (matmul operands bf16, accumulation fp32 in PSUM):
  1. LN(xs rows) -> bf16, PE-transpose -> xsT [dim, 256]; qT = Wq'.T @ xsT
     (LN scale folded into W, q pre-scaled by d^-1/2).
  2. AllGather ctx -> ctx_g [2048, dim] fp16; LN -> ctxT [dim, 2048].
  3. kT = Wk'.T @ ctxT -> [inner, 2048]; V = ctxT.T @ Wv' -> [2048, inner];
     V_ext per (j, h): 66-col slot [V_h*mask | mask | pad] -> masking and
     the softmax denominator both come free out of the AV matmul.
  4. Attention per (head h, m-chunk j):
       simT[m128, 256] = kT_hj.T @ qT_h   (PE)
       pT = exp(simT)  (ACT, PSUM->SBUF bf16; logits ~N(0,1) after LN so
                        exp cannot overflow without max-subtraction)
       av[q128, 65] += pT.T @ V_ext_jh    (PE accumulation)
     plus null token: s0 = k_null.T @ qT_h, e0 = exp(s0),
     av += e0.T @ [v_null | 1] (rank-1 matmul opens the PSUM bank).
     r = 1/av[:,64]; attn_out[:, h*64:] = av[:, :64] * r.
  5. out = attn_outT @ Wo + bo -> fp16 -> DMA to DRAM.
"""

import threading
import zlib

import numpy as np
import ml_dtypes

import jax
import jax.numpy as jnp
from jax.sharding import Mesh, PartitionSpec, NamedSharding
from jax.experimental.shard_map import shard_map

import concourse.bass as bass
import concourse.mybir as mybir
import concourse.tile as tile
from concourse import bacc
from concourse.masks import make_identity

F32 = mybir.dt.float32
F16 = mybir.dt.float16
BF16 = mybir.dt.bfloat16

P = 128
DIM = 1024
HEADS = 8
DH = 64
INNER = 512
B = 4
N = 2048          # query rows per batch
M = 2048          # context rows per batch
NQR = 256         # query rows per core per stage
NCR = 256         # ctx rows contributed per core per stage
NQ = NQR // P     # 2 query chunks
NJ = M // P       # 16 context chunks
KC = DIM // P     # 8 contraction chunks
EPS = 1e-6
NCORES = 8

_RT = {}


def build_program():
    nc = bacc.Bacc(None, target_bir_lowering=False, num_devices=NCORES)

    xc_d = nc.dram_tensor("xc", [NQR, 2 * DIM], F16, kind="ExternalInput")
    maskm_d = nc.dram_tensor("maskm", [P, NJ], F32, kind="ExternalInput")
    wq_d = nc.dram_tensor("wq", [DIM, INNER], BF16, kind="ExternalInput")
    wk_d = nc.dram_tensor("wk", [DIM, INNER], BF16, kind="ExternalInput")
    wv_d = nc.dram_tensor("wv", [DIM, INNER], BF16, kind="ExternalInput")
    wo_d = nc.dram_tensor("wo", [INNER, DIM], BF16, kind="ExternalInput")
    bq_d = nc.dram_tensor("bq", [1, INNER], BF16, kind="ExternalInput")
    bk_d = nc.dram_tensor("bk", [1, INNER], BF16, kind="ExternalInput")
    vb_d = nc.dram_tensor("vb", [1, INNER], BF16, kind="ExternalInput")
    bo_d = nc.dram_tensor("bo", [1, DIM], BF16, kind="ExternalInput")
    knull_d = nc.dram_tensor("knull", [P, 1], BF16, kind="ExternalInput")
    vne_d = nc.dram_tensor("vne", [1, 66], BF16, kind="ExternalInput")
    out_d = nc.dram_tensor("out", [NQR, DIM], F16, kind="ExternalOutput")

    with tile.TileContext(nc) as tc:
        with (
            tc.tile_pool(name="dram", bufs=1, space="DRAM") as dram,
            tc.tile_pool(name="consts", bufs=1) as consts,
            tc.tile_pool(name="persist", bufs=1) as persist,
            tc.tile_pool(name="lnio", bufs=3) as lnio,
            tc.tile_pool(name="lnbf", bufs=3) as lnbf,
            tc.tile_pool(name="lntmp", bufs=4) as lntmp,
            tc.tile_pool(name="ptp", bufs=3) as ptp,
            tc.tile_pool(name="e0p", bufs=2) as e0p,
            tc.tile_pool(name="rp", bufs=2) as rp,
            tc.tile_pool(name="aop", bufs=2) as aop,
            tc.tile_pool(name="outp", bufs=2) as outp,
            tc.tile_pool(name="ps", bufs=2, space="PSUM") as psp,
            tc.tile_pool(name="sim", bufs=2, space="PSUM") as simp,
            tc.tile_pool(name="av", bufs=2, space="PSUM") as avp,
        ):
            # ---- ctx AllGather (gpsimd, DRAM bounce) ----
            ctx_bounce = dram.tile([NCR, DIM], F16, tag="ctxb")
            ctx_g = dram.tile([M, DIM], F16, tag="ctxg")
            nc.gpsimd.dma_start(ctx_bounce[:], xc_d[:, DIM:2 * DIM])
            nc.gpsimd.collective_compute(
                "AllGather",
                mybir.AluOpType.bypass,
                replica_groups=[list(range(NCORES))],
                ins=[ctx_bounce.opt()],
                outs=[ctx_g.opt()],
            )

            # ---- constants ----
            wq_sb = consts.tile([P, KC, INNER], BF16, tag="wq")
            nc.sync.dma_start(out=wq_sb, in_=wq_d.rearrange("(kc p) m -> p kc m", p=P))
            wk_sb = consts.tile([P, KC, INNER], BF16, tag="wk")
            nc.sync.dma_start(out=wk_sb, in_=wk_d.rearrange("(kc p) m -> p kc m", p=P))
            wv_sb = consts.tile([P, KC, INNER], BF16, tag="wv")
            nc.sync.dma_start(out=wv_sb, in_=wv_d.rearrange("(kc p) m -> p kc m", p=P))
            wo_sb = consts.tile([P, 4, DIM], BF16, tag="wo")
            nc.sync.dma_start(out=wo_sb, in_=wo_d.rearrange("(ic p) n -> p ic n", p=P))
            bqr_sb = consts.tile([1, INNER], BF16, tag="bqr")
            nc.sync.dma_start(out=bqr_sb, in_=bq_d[:])
            bkr_sb = consts.tile([1, INNER], BF16, tag="bkr")
            nc.sync.dma_start(out=bkr_sb, in_=bk_d[:])
            bvr_sb = consts.tile([1, INNER], BF16, tag="bvr")
            nc.sync.dma_start(out=bvr_sb, in_=vb_d[:])
            bor_sb = consts.tile([1, DIM], BF16, tag="bor")
            nc.sync.dma_start(out=bor_sb, in_=bo_d[:])
            ones_row = consts.tile([1, 512], BF16, tag="ones_row")
            nc.vector.memset(ones_row, 1.0)
            maskm_sb = consts.tile([P, NJ], F32, tag="maskm")
            nc.sync.dma_start(out=maskm_sb, in_=maskm_d[:])
            knull_sb = consts.tile([P, 1], BF16, tag="knull")
            nc.sync.dma_start(out=knull_sb, in_=knull_d[:])
            vne_sb = consts.tile([1, 66], BF16, tag="vne")
            nc.sync.dma_start(out=vne_sb, in_=vne_d[:])
            ident = consts.tile([P, P], BF16, tag="ident")
            make_identity(nc, ident)
            eps_sb = consts.tile([P, 1], F32, tag="eps")
            nc.vector.memset(eps_sb, EPS)

            # ---- persistent activations ----
            ctxT = persist.tile([P, KC, M], BF16, tag="ctxT")
            xsT = persist.tile([P, KC, NQR], BF16, tag="xsT")
            kT = persist.tile([P, 4, M], BF16, tag="kT")
            vext = persist.tile([P, NJ, HEADS, 66], BF16, tag="vext")
            qT = persist.tile([P, 4, NQR], BF16, tag="qT")
            attn_out = persist.tile([P, NQ, INNER], BF16, tag="attn_out")

            def ln_transpose(src, n_rows, dstT, col0=0):
                for j in range(n_rows // P):
                    xt = lnio.tile([P, DIM], F16, tag="xt")
                    nc.sync.dma_start(
                        out=xt, in_=src[j * P:(j + 1) * P, col0:col0 + DIM])
                    stats = lntmp.tile([P, 2, 6], F32, tag="stats")
                    nc.vector.bn_stats(out=stats[:, 0, :], in_=xt[:, 0:512])
                    nc.vector.bn_stats(out=stats[:, 1, :], in_=xt[:, 512:1024])
                    mv = lntmp.tile([P, 2], F32, tag="mv")
                    nc.vector.bn_aggr(out=mv, in_=stats)
                    rstd = lntmp.tile([P, 1], F32, tag="rstd")
                    nc.scalar.activation(out=rstd, in_=mv[:, 1:2],
                                         func=mybir.ActivationFunctionType.Sqrt,
                                         bias=eps_sb)
                    nc.vector.reciprocal(out=rstd, in_=rstd)
                    xn = lnbf.tile([P, DIM], BF16, tag="xn")
                    nc.vector.tensor_scalar(
                        out=xn, in0=xt, scalar1=mv[:, 0:1], scalar2=rstd,
                        op0=mybir.AluOpType.subtract, op1=mybir.AluOpType.mult)
                    tp = psp.tile([P, KC * P], BF16, tag="ps")
                    for i in range(KC):
                        nc.tensor.transpose(out=tp[:, i * P:(i + 1) * P],
                                            in_=xn[:, i * P:(i + 1) * P],
                                            identity=ident)
                    for i in range(KC):
                        nc.scalar.copy(out=dstT[:, i, j * P:(j + 1) * P],
                                       in_=tp[:, i * P:(i + 1) * P])

            # queries first: independent of the collective
            ln_transpose(xc_d, NQR, xsT)

            # ---- q projection: [inner, 256] ----
            for ic in range(4):
                ps = psp.tile([P, 512], F32, tag="ps")
                for kc in range(KC):
                    nc.tensor.matmul(
                        out=ps[:, 0:NQR],
                        lhsT=wq_sb[:, kc, ic * P:(ic + 1) * P],
                        rhs=xsT[:, kc, :],
                        start=(kc == 0), stop=False)
                nc.tensor.matmul(
                    out=ps[:, 0:NQR], lhsT=bqr_sb[:, ic * P:(ic + 1) * P],
                    rhs=ones_row[:, 0:NQR], start=False, stop=True)
                nc.vector.tensor_copy(out=qT[:, ic, :], in_=ps[:, 0:NQR])

            # ---- gathered context: LN + transpose ----
            ln_transpose(ctx_g, M, ctxT)

            # ---- kT projection: [inner, m] ----
            for ic in range(4):
                for mh in range(4):
                    ps = psp.tile([P, 512], F32, tag="ps")
                    for kc in range(KC):
                        nc.tensor.matmul(
                            out=ps,
                            lhsT=wk_sb[:, kc, ic * P:(ic + 1) * P],
                            rhs=ctxT[:, kc, mh * 512:(mh + 1) * 512],
                            start=(kc == 0), stop=False)
                    nc.tensor.matmul(
                        out=ps, lhsT=bkr_sb[:, ic * P:(ic + 1) * P],
                        rhs=ones_row, start=False, stop=True)
                    nc.vector.tensor_copy(
                        out=kT[:, ic, mh * 512:(mh + 1) * 512], in_=ps)

            # ---- V projection (natural layout) + mask/bias -> V_ext ----
            for j in range(NJ):
                ps = psp.tile([P, 512], F32, tag="ps")
                for kc in range(KC):
                    nc.tensor.matmul(
                        out=ps,
                        lhsT=ctxT[:, kc, j * P:(j + 1) * P],
                        rhs=wv_sb[:, kc, :],
                        start=(kc == 0), stop=False)
                nc.tensor.matmul(
                    out=ps, lhsT=ones_row[:, 0:P], rhs=bvr_sb,
                    start=False, stop=True)
                for h in range(HEADS):
                    nc.vector.tensor_scalar_mul(
                        out=vext[:, j, h, 0:64],
                        in0=ps[:, h * 64:(h + 1) * 64],
                        scalar1=maskm_sb[:, j:j + 1])
                # denominator column: mask value (0/1) per row
                for h in range(HEADS):
                    nc.scalar.copy(out=vext[:, j, h, 64:65],
                                   in_=maskm_sb[:, j:j + 1])

            # ---- attention ----
            for h in range(HEADS):
                hp = (h % 2) * DH
                ic = h // 2
                qh = qT[hp:hp + DH, ic, :]
                # null-token logits s0[1, 256] and e0 = exp(s0)
                s0 = psp.tile([1, 512], F32, tag="s0")
                nc.tensor.matmul(out=s0[:, 0:NQR], lhsT=knull_sb[hp:hp + DH, :],
                                 rhs=qh, start=True, stop=True)
                e0 = e0p.tile([1, NQR], BF16, tag="e0")
                nc.scalar.activation(out=e0, in_=s0[:, 0:NQR],
                                     func=mybir.ActivationFunctionType.Exp)
                # av [P, 4, P] f32 = exactly one 2KB PSUM bank; slots 0..1
                # used. start=True on the first (null) matmul zeroes the
                # bank; all later matmuls accumulate (bank-granular
                # bookkeeping, hence skip_group_check).
                av = avp.tile([P, 4, P], F32, tag="av")
                for q2 in range(NQ):
                    nc.tensor.matmul(
                        out=av[:, q2, 0:65],
                        lhsT=e0[:, q2 * P:(q2 + 1) * P],
                        rhs=vne_sb[:, 0:65],
                        start=(q2 == 0), stop=False,
                        skip_group_check=True)
                for j in range(NJ):
                    sm = simp.tile([P, 512], F32, tag="sim")
                    kh = kT[hp:hp + DH, ic, j * P:(j + 1) * P]
                    nc.tensor.matmul(out=sm[:, 0:NQR], lhsT=kh, rhs=qh,
                                     start=True, stop=True)
                    pt = ptp.tile([P, NQR], BF16, tag="pt")
                    nc.scalar.activation(out=pt, in_=sm[:, 0:NQR],
                                         func=mybir.ActivationFunctionType.Exp)
                    for q2 in range(NQ):
                        nc.tensor.matmul(
                            out=av[:, q2, 0:65],
                            lhsT=pt[:, q2 * P:(q2 + 1) * P],
                            rhs=vext[:, j, h, 0:65],
                            start=False, stop=(j == NJ - 1 and q2 == NQ - 1),
                            skip_group_check=True)
                r = rp.tile([P, NQ], F32, tag="r")
                for q2 in range(NQ):
                    nc.vector.reciprocal(out=r[:, q2:q2 + 1],
                                         in_=av[:, q2, 64:65])
                for q2 in range(NQ):
                    nc.vector.tensor_scalar_mul(
                        out=attn_out[:, q2, h * DH:(h + 1) * DH],
                        in0=av[:, q2, 0:64], scalar1=r[:, q2:q2 + 1])

            # ---- output projection ----
            for q2 in range(NQ):
                tp = psp.tile([P, KC * P], BF16, tag="ps")
                for i in range(4):
                    nc.tensor.transpose(out=tp[:, i * P:(i + 1) * P],
                                        in_=attn_out[:, q2, i * P:(i + 1) * P],
                                        identity=ident)
                aoT = aop.tile([P, 4 * P], BF16, tag="aoT")
                nc.vector.tensor_copy(out=aoT, in_=tp[:, 0:4 * P])
                ot = outp.tile([P, DIM], F16, tag="ot")
                for oh in range(2):
                    ps = psp.tile([P, 512], F32, tag="ps")
                    for ic in range(4):
                        nc.tensor.matmul(
                            out=ps, lhsT=aoT[:, ic * P:(ic + 1) * P],
                            rhs=wo_sb[:, ic, oh * 512:(oh + 1) * 512],
                            start=(ic == 0), stop=False)
                    nc.tensor.matmul(
                        out=ps, lhsT=ones_row[:, 0:P],
                        rhs=bor_sb[:, oh * 512:(oh + 1) * 512],
                        start=False, stop=True)
                    nc.vector.tensor_copy(
                        out=ot[:, oh * 512:(oh + 1) * 512], in_=ps)
                nc.sync.dma_start(out=out_d[q2 * P:(q2 + 1) * P, :], in_=ot)

    nc.compile()
    return nc


def _build_runtime():
    from concourse.bass2jax import (
        _bass_exec_p, install_neuronx_cc_hook, partition_id_tensor)

    nc = build_program()
    install_neuronx_cc_hook()
    partition_name = nc.partition_id_tensor.name if nc.partition_id_tensor else None
    in_names, out_names, out_avals = [], [], []
    for alloc in nc.m.functions[0].allocations:
        if not isinstance(alloc, mybir.MemoryLocationSet):
            continue
        name = alloc.memorylocations[0].name
        if alloc.kind == "ExternalInput":
            if name != partition_name:
                in_names.append(name)
        elif alloc.kind == "ExternalOutput":
            out_names.append(name)
            out_avals.append(jax.core.ShapedArray(
                tuple(alloc.tensor_shape), mybir.dt.np(alloc.dtype)))
    n_params = len(in_names)
    n_outs = len(out_avals)
    in_names_all = in_names + out_names + (
        [partition_name] if partition_name else [])
    donate = tuple(range(n_params, n_params + n_outs))

    def _body(*args):
        operands = list(args)
        if partition_name is not None:
            operands.append(partition_id_tensor())
        return tuple(_bass_exec_p.bind(
            *operands, out_avals=tuple(out_avals), in_names=tuple(in_names_all),
            out_names=tuple(out_names), lowering_input_output_aliases=(),
            sim_require_finite=True, sim_require_nnan=True, nc=nc))

    devices = jax.devices()[:NCORES]
    mesh = Mesh(np.asarray(devices), ("core",))
    S = NamedSharding(mesh, PartitionSpec("core"))
    sharded = jax.jit(
        shard_map(_body, mesh=mesh,
                  in_specs=(PartitionSpec("core"),) * (n_params + n_outs),
                  out_specs=(PartitionSpec("core"),) * n_outs,
                  check_rep=False),
        donate_argnums=donate, keep_unused=True)

    zeros_jit = jax.jit(
        lambda: tuple(
            jnp.zeros((NCORES * a.shape[0], *a.shape[1:]), a.dtype)
            for a in out_avals),
        out_shardings=tuple([S] * n_outs))

    return dict(nc=nc, sharded=sharded, zeros_jit=zeros_jit,
                in_names=in_names, sharding=S)


def _fold_weights(ln_x_scale, ln_x_bias, ln_c_scale, ln_c_bias,
                  Wq, bq, Wkv, bkv, Wo, bo, null_kv):
    f32 = np.float32
    bf16 = ml_dtypes.bfloat16
    scale = np.float32(DH ** (-0.5))
    Wq = np.asarray(Wq, f32)
    Wkv = np.asarray(Wkv, f32)
    Wo = np.asarray(Wo, f32)
    ln_x_scale = np.asarray(ln_x_scale, f32)
    ln_x_bias = np.asarray(ln_x_bias, f32)
    ln_c_scale = np.asarray(ln_c_scale, f32)
    ln_c_bias = np.asarray(ln_c_bias, f32)
    bq = np.asarray(bq, f32)
    bkv = np.asarray(bkv, f32)
    bo = np.asarray(bo, f32)
    null_kv = np.asarray(null_kv, f32)

    wq_f = (ln_x_scale[:, None] * Wq) * scale
    bq_f = (ln_x_bias @ Wq + bq) * scale
    wkv_f = ln_c_scale[:, None] * Wkv
    bkv_f = ln_c_bias @ Wkv + bkv
    wk_f, wv_f = wkv_f[:, :INNER], wkv_f[:, INNER:]
    bk_f, bv_f = bkv_f[:INNER], bkv_f[INNER:]

    return {
        "wq": np.ascontiguousarray(wq_f.astype(bf16)),
        "wk": np.ascontiguousarray(wk_f.astype(bf16)),
        "wv": np.ascontiguousarray(wv_f.astype(bf16)),
        "wo": np.ascontiguousarray(Wo.astype(bf16)),
        "bq": np.ascontiguousarray(bq_f.reshape(1, INNER).astype(bf16)),
        "bk": np.ascontiguousarray(bk_f.reshape(1, INNER).astype(bf16)),
        "vb": np.ascontiguousarray(bv_f.reshape(1, INNER).astype(bf16)),
        "bo": np.ascontiguousarray(bo.reshape(1, DIM).astype(bf16)),
        "knull": np.ascontiguousarray(
            np.tile(null_kv[0], 2).reshape(P, 1).astype(bf16)),
        "vne": np.ascontiguousarray(
            np.concatenate([null_kv[1], [1.0, 0.0]]).reshape(1, 66).astype(bf16)),
    }


def _weights_key(*arrs):
    h = 0
    for a in arrs:
        a = np.ascontiguousarray(a)
        h = zlib.adler32(a.view(np.uint8).reshape(-1), h)
    return h


def kernel(**inputs):
    if "rt" not in _RT:
        _RT["rt"] = _build_runtime()
    rt = _RT["rt"]
    S = rt["sharding"]

    x = np.asarray(inputs["x"], np.float32)
    context = np.asarray(inputs["context"], np.float32)
    mask = np.asarray(inputs["mask"])

    wkey = _weights_key(
        np.asarray(inputs["Wq"], np.float32), np.asarray(inputs["Wkv"], np.float32),
        np.asarray(inputs["Wo"], np.float32), np.asarray(inputs["bq"], np.float32),
        np.asarray(inputs["bkv"], np.float32), np.asarray(inputs["bo"], np.float32),
        np.asarray(inputs["ln_x_scale"], np.float32), np.asarray(inputs["ln_x_bias"], np.float32),
        np.asarray(inputs["ln_c_scale"], np.float32), np.asarray(inputs["ln_c_bias"], np.float32),
        np.asarray(inputs["null_kv"], np.float32))
    if _RT.get("wkey") != wkey:
        shared = _fold_weights(
            inputs["ln_x_scale"], inputs["ln_x_bias"], inputs["ln_c_scale"],
            inputs["ln_c_bias"], inputs["Wq"], inputs["bq"], inputs["Wkv"],
            inputs["bkv"], inputs["Wo"], inputs["bo"], inputs["null_kv"])
        dev_w = {}
        for name, arr in shared.items():
            cat = np.ascontiguousarray(
                np.tile(arr, (NCORES,) + (1,) * (arr.ndim - 1)))
            dev_w[name] = jax.device_put(cat, S)
        jax.block_until_ready(list(dev_w.values()))
        _RT["dev_w"] = dev_w
        _RT["wkey"] = wkey
    dev_w = _RT["dev_w"]

    # one fp16 slab per stage: [2048, 2048] = [x[b] | context[b]] so each
    # core's shard is [256, 2048]; the f16 cast happens during the fill
    slabs = []
    for b in range(B):
        xc = np.empty((N, 2 * DIM), np.float16)
        xc[:, :DIM] = x[b]
        xc[:, DIM:] = context[b]
        slabs.append(xc)
    masks = [
        np.ascontiguousarray(
            np.tile(mask[b].astype(np.float32).reshape(NJ, P).T, (NCORES, 1)))
        for b in range(B)
    ]
    # single submission; masks first so stage 0 is not blocked by later slabs
    dev = jax.device_put(masks + slabs, [S] * (2 * B))
    d_masks, d_slabs = dev[:B], dev[B:]

    out = np.empty((B, N, DIM), np.float32)
    threads = []

    def fetch(b, arr):
        out[b] = np.asarray(arr).astype(np.float32)

    in_names = rt["in_names"]
    for b in range(B):
        stage_in = {"xc": d_slabs[b], "maskm": d_masks[b], **dev_w}
        args = [stage_in[name] for name in in_names]
        zeros = rt["zeros_jit"]()
        outs = rt["sharded"](*args, *zeros)
        th = threading.Thread(target=fetch, args=(b, outs[0]))
        th.start()
        threads.append(th)

    for th in threads:
        th.join()
    return out
